# revision 7
# baseline (speedup 1.0000x reference)
"""Bahdanau attention kernel for Trainium2, data-parallel over batch on 8 cores.

Shapes (hardcoded): B=64, S=512, H=1024, AH=512. Each core handles 8 batches.

Math per batch b:
  en_proj[s,o] = sum_h enhy[b,s,h] * W_en[o,h]            (big matmul, fp16)
  agg[s,o]     = en_proj + de_proj[b,o] + b_* + past[b,s]*W_cv[o]
  score[s]     = sum_o tanh(agg[s,o]) * W_out[o]
  attn         = softmax_s(score);  h_attn[h] = sum_s attn[s]*enhy[b,s,h]

Device layout: agg is computed transposed ([o on partitions, s free]) so the
W_out contraction is a matmul; de_proj+bias rides in as the tanh bias
([P,1] per (o-chunk, b)); the rank-1 past*W_cv term is one fused DVE op.
"""

import numpy as np

B, S, H = 64, 512, 1024
N_CORES = 8
BL = B // N_CORES  # batches per core = 8
P = 128
HC = H // P        # h chunks = 8
OC = H // P        # o chunks = 8
SC = S // P        # s chunks = 4
NH = H // 512      # 512-wide halves of H = 2

_PROGRAM = None


def _build_program():
    import concourse.bass as bass
    import concourse.mybir as mybir
    import concourse.tile as tile
    from concourse import bacc

    f32 = mybir.dt.float32
    f16 = mybir.dt.float16
    Alu = mybir.AluOpType
    Act = mybir.ActivationFunctionType

    nc = bacc.Bacc("TRN2", target_bir_lowering=False, debug=False,
                   num_devices=N_CORES)

    enhy_d = nc.dram_tensor("enhy_loc", [BL, S, H], f32, kind="ExternalInput").ap()
    ldehy_d = nc.dram_tensor("last_dehy_loc", [BL, H], f32, kind="ExternalInput").ap()
    past_d = nc.dram_tensor("past_loc", [BL, S], f32, kind="ExternalInput").ap()
    wen_d = nc.dram_tensor("w_enT16", [HC, P, H], f16, kind="ExternalInput").ap()
    wde_d = nc.dram_tensor("w_deT16", [HC, P, H], f16, kind="ExternalInput").ap()
    wcv_d = nc.dram_tensor("w_cvT", [P, OC], f32, kind="ExternalInput").ap()
    wout_d = nc.dram_tensor("w_outT16", [P, OC], f16, kind="ExternalInput").ap()
    bvec_d = nc.dram_tensor("bvecT", [P, OC], f32, kind="ExternalInput").ap()
    id32_d = nc.dram_tensor("id32", [P, BL], f32, kind="ExternalInput").ap()
    id16_d = nc.dram_tensor("id16", [P, BL], f16, kind="ExternalInput").ap()

    hattn_d = nc.dram_tensor("h_attn_loc", [BL, H], f32, kind="ExternalOutput").ap()
    attn_d = nc.dram_tensor("attn_loc", [BL, S], f32, kind="ExternalOutput").ap()

    with tile.TileContext(nc) as tc:
        with (
            tc.tile_pool(name="consts", bufs=1) as consts,
            tc.tile_pool(name="e16s", bufs=1) as e16s,
            tc.tile_pool(name="e32p", bufs=6) as e32p,
            tc.tile_pool(name="eTp", bufs=3) as eTp,
            tc.tile_pool(name="pbp", bufs=2) as pbp,
            tc.tile_pool(name="wtp", bufs=4) as wtp,
            tc.tile_pool(name="ps_agg", bufs=3, space="PSUM") as ps_agg,
            tc.tile_pool(name="ps_score", bufs=2, space="PSUM") as ps_score,
            tc.tile_pool(name="ps_misc", bufs=2, space="PSUM") as ps_misc,
        ):
            # ---- constants in ----
            wen_sb = consts.tile([P, HC, H], f16)
            nc.sync.dma_start(wen_sb[:], wen_d.rearrange("t p o -> p t o"))
            wde_sb = consts.tile([P, HC, H], f16)
            nc.sync.dma_start(wde_sb[:], wde_d.rearrange("t p o -> p t o"))
            wcv_sb = consts.tile([P, OC], f32)
            nc.sync.dma_start(wcv_sb[:], wcv_d)
            wout_sb = consts.tile([P, OC], f16)
            nc.sync.dma_start(wout_sb[:], wout_d)
            bvec_sb = consts.tile([P, OC], f32)
            nc.sync.dma_start(bvec_sb[:], bvec_d)
            id32_sb = consts.tile([P, BL], f32)
            nc.sync.dma_start(id32_sb[:], id32_d)
            id16_sb = consts.tile([P, BL], f16)
            nc.sync.dma_start(id16_sb[:], id16_d)

            # ---- last_dehy^T (pad to 128 partitions, transpose via identity mm) ----
            ldpad = consts.tile([P, H], f32)
            nc.vector.memset(ldpad[:], 0.0)
            nc.sync.dma_start(ldpad[0:BL, :], ldehy_d)
            ldT = consts.tile([P, HC, BL], f16)  # [h_in, h_out, b]
            for t in range(HC):
                ps = ps_misc.tile([P, BL], f32, tag="m")
                nc.tensor.matmul(ps[:], ldpad[:, t * P:(t + 1) * P], id32_sb[:],
                                 start=True, stop=True)
                nc.vector.tensor_copy(ldT[:, t, :], ps[:])

            # ---- de_proj = last_dehy @ W_de.T  -> [BL, H] ----
            depad = consts.tile([P, H], f32)
            nc.vector.memset(depad[:], 0.0)
            for oh in range(NH):
                psd = ps_misc.tile([BL, 512], f32, tag="m")
                for t in range(HC):
                    nc.tensor.matmul(psd[:], ldT[:, t, :],
                                     wde_sb[:, t, oh * 512:(oh + 1) * 512],
                                     start=(t == 0), stop=(t == HC - 1))
                nc.vector.tensor_copy(depad[0:BL, oh * 512:(oh + 1) * 512], psd[:])

            # ---- dvecT[o_in, o_out, b] = de_proj^T + (b_en+b_de+b_cv) ----
            dvecT = consts.tile([P, OC, BL], f32)
            for t in range(OC):
                ps = ps_misc.tile([P, BL], f32, tag="m")
                nc.tensor.matmul(ps[:], depad[:, t * P:(t + 1) * P], id32_sb[:],
                                 start=True, stop=True)
                nc.vector.tensor_scalar_add(dvecT[:, t, :], ps[:],
                                            bvec_sb[:, t:t + 1])

            # ---- per-batch main pipeline ----
            e16_tiles = []
            scores_sb = consts.tile([BL, S], f32)
            for b in range(BL):
                e16 = e16s.tile([P, SC, H], f16, tag=f"e16_{b}")
                e16_tiles.append(e16)
                eT = eTp.tile([P, HC, S], f16)  # [h_in, h_out, s]
                for so in range(SC):
                    # load one s-chunk, cast on DVE/ACT alternately, then
                    # XBAR-transpose it, alternating the issuing DGE engine
                    e32c = e32p.tile([P, H], f32)
                    nc.sync.dma_start(
                        e32c[:], enhy_d[b, so * P:(so + 1) * P, :])
                    if so % 2 == 0:
                        nc.vector.tensor_copy(e16[:, so, :], e32c[:])
                    else:
                        nc.scalar.copy(e16[:, so, :], e32c[:])
                    nc.sync.dma_start_transpose(
                        eT[:, :, so * P:(so + 1) * P], e16[:, so, :])

                pb = pbp.tile([P, S], f32)
                nc.sync.dma_start(pb[:], past_d[b].partition_broadcast(P))

                score_ps = ps_score.tile([1, S], f32)
                for oc in range(OC):
                    ps = ps_agg.tile([P, S], f32)
                    for hc in range(HC):
                        nc.tensor.matmul(ps[:], wen_sb[:, hc, oc * P:(oc + 1) * P],
                                         eT[:, hc, :],
                                         start=(hc == 0), stop=(hc == HC - 1))
                    # agg += past[b,s] * W_cv[o]
                    nc.vector.scalar_tensor_tensor(
                        out=ps[:], in0=pb[:], scalar=wcv_sb[:, oc:oc + 1],
                        in1=ps[:], op0=Alu.mult, op1=Alu.add)
                    # wt = tanh(agg + de_proj[b,o] + bias[o])
                    wt = wtp.tile([P, S], f16)
                    nc.scalar.activation(out=wt[:], in_=ps[:], func=Act.Tanh,
                                         bias=dvecT[:, oc, b:b + 1], scale=1.0)
                    # score += W_out[o-chunk] . wt
                    nc.tensor.matmul(score_ps[:], wout_sb[:, oc:oc + 1], wt[:],
                                     start=(oc == 0), stop=(oc == OC - 1),
                                     skip_group_check=True)
                srow = wtp.tile([1, S], f32, tag="srow")
                nc.vector.tensor_copy(srow[:], score_ps[:])
                nc.sync.dma_start(scores_sb[b:b + 1, :], srow[:])

            # ---- softmax over s for all batches at once ([BL, S]) ----
            mx = consts.tile([BL, 1], f32)
            nc.vector.tensor_reduce(mx[:], scores_sb[:], axis=mybir.AxisListType.X,
                                    op=Alu.max, negate=True)
            ex = consts.tile([BL, S], f32)
            sm = consts.tile([BL, 1], f32)
            nc.scalar.activation(out=ex[:], in_=scores_sb[:], func=Act.Exp,
                                 bias=mx[:], scale=1.0, accum_out=sm[:])
            rec = consts.tile([BL, 1], f32)
            nc.vector.reciprocal(rec[:], sm[:])
            attn32 = consts.tile([BL, S], f32)
            nc.vector.tensor_scalar_mul(attn32[:], ex[:], rec[:])
            nc.sync.dma_start(attn_d[:], attn32[:])

            # ---- attn^T (fp16) for the final contraction ----
            atpad = consts.tile([P, S], f16)
            nc.vector.memset(atpad[:], 0.0)
            nc.vector.tensor_copy(atpad[0:BL, :], attn32[:])
            attnT = consts.tile([P, SC, BL], f16)  # [s_in, s_out, b]
            for sc in range(SC):
                ps = ps_misc.tile([P, BL], f32, tag="m")
                nc.tensor.matmul(ps[:], atpad[:, sc * P:(sc + 1) * P], id16_sb[:],
                                 start=True, stop=True)
                nc.vector.tensor_copy(attnT[:, sc, :], ps[:])

            # ---- h_attn[b,h] = sum_s attn[b,s] * enhy[b,s,h] ----
            for b in range(BL):
                for hh in range(NH):
                    hp = ps_misc.tile([1, 512], f32, tag="m")
                    for sc in range(SC):
                        nc.tensor.matmul(hp[:], attnT[:, sc, b:b + 1],
                                         e16_tiles[b][:, sc, hh * 512:(hh + 1) * 512],
                                         start=(sc == 0), stop=(sc == SC - 1),
                                         skip_group_check=True)
                    hrow = wtp.tile([1, 512], f32, tag="hrow")
                    nc.vector.tensor_copy(hrow[:], hp[:])
                    nc.sync.dma_start(hattn_d[b:b + 1, hh * 512:(hh + 1) * 512],
                                      hrow[:])

    nc.compile()
    return nc


def _get_program():
    global _PROGRAM
    if _PROGRAM is None:
        _PROGRAM = _build_program()
    return _PROGRAM


def _host_prep(W_en, b_en, W_de, b_de, W_cv, b_cv, W_out):
    """One-time weight layout transforms (transpose/cast/reshape)."""
    w_enT16 = np.ascontiguousarray(
        W_en.T.astype(np.float16).reshape(HC, P, H))
    w_deT16 = np.ascontiguousarray(
        W_de.T.astype(np.float16).reshape(HC, P, H))
    w_cvT = np.ascontiguousarray(W_cv[:, 0].reshape(OC, P).T.astype(np.float32))
    w_outT16 = np.ascontiguousarray(W_out[0].reshape(OC, P).T.astype(np.float16))
    bvecT = np.ascontiguousarray(
        (b_en + b_de + b_cv).reshape(OC, P).T.astype(np.float32))
    id32 = np.eye(P, BL, dtype=np.float32)
    id16 = np.eye(P, BL, dtype=np.float16)
    return dict(w_enT16=w_enT16, w_deT16=w_deT16, w_cvT=w_cvT,
                w_outT16=w_outT16, bvecT=bvecT, id32=id32, id16=id16)


def make_in_maps(inputs):
    enhy = np.ascontiguousarray(np.asarray(inputs["enhy"], dtype=np.float32))
    last_dehy = np.ascontiguousarray(np.asarray(inputs["last_dehy"], np.float32))
    past_attn = np.ascontiguousarray(np.asarray(inputs["past_attn"], np.float32))
    w = _host_prep(*(np.asarray(inputs[k], np.float32) for k in
                     ("W_en", "b_en", "W_de", "b_de", "W_cv", "b_cv", "W_out")))
    in_maps = []
    for c in range(N_CORES):
        sl = slice(c * BL, (c + 1) * BL)
        in_maps.append({
            "enhy_loc": enhy[sl],
            "last_dehy_loc": last_dehy[sl],
            "past_loc": past_attn[sl],
            **w,
        })
    return in_maps


def kernel(last_dehy, enhy, past_attn, hidden_attn,
           W_en, b_en, W_de, b_de, W_cv, b_cv, W_out):
    from concourse.bass_utils import run_bass_kernel_spmd

    nc = _get_program()
    in_maps = make_in_maps(dict(
        last_dehy=last_dehy, enhy=enhy, past_attn=past_attn,
        W_en=W_en, b_en=b_en, W_de=W_de, b_de=b_de,
        W_cv=W_cv, b_cv=b_cv, W_out=W_out))
    res = run_bass_kernel_spmd(nc, in_maps, core_ids=list(range(N_CORES)))
    h_attn = np.concatenate([res.results[c]["h_attn_loc"] for c in range(N_CORES)])
    attn = np.concatenate([res.results[c]["attn_loc"] for c in range(N_CORES)])
    hidden = np.array(np.asarray(hidden_attn, np.float32))
    return (h_attn, attn, hidden)


# revision 9
# speedup vs baseline: 1.3178x; 1.3178x over previous
"""Bahdanau attention kernel for Trainium2, data-parallel over batch on 8 cores.

Shapes (hardcoded): B=64, S=512, H=1024, AH=512. Each core handles 8 batches.

Math per batch b:
  en_proj[s,o] = sum_h enhy[b,s,h] * W_en[o,h]            (big matmul, fp16)
  agg[s,o]     = en_proj + de_proj[b,o] + b_* + past[b,s]*W_cv[o]
  score[s]     = sum_o tanh(agg[s,o]) * W_out[o]
  attn         = softmax_s(score);  h_attn[h] = sum_s attn[s]*enhy[b,s,h]

Device layout: agg is computed transposed ([o on partitions, s free]) so the
W_out contraction is a matmul; de_proj+bias rides in as the tanh bias
([P,1] per (o-chunk, b)); the rank-1 past*W_cv term is one fused DVE op.
"""

import numpy as np

B, S, H = 64, 512, 1024
N_CORES = 8
BL = B // N_CORES  # batches per core = 8
P = 128
HC = H // P        # h chunks = 8
OC = H // P        # o chunks = 8
SC = S // P        # s chunks = 4
NH = H // 512      # 512-wide halves of H = 2

_PROGRAM = None


def _build_program():
    import concourse.bass as bass
    import concourse.mybir as mybir
    import concourse.tile as tile
    from concourse import bacc

    f32 = mybir.dt.float32
    f16 = mybir.dt.float16
    Alu = mybir.AluOpType
    Act = mybir.ActivationFunctionType

    nc = bacc.Bacc("TRN2", target_bir_lowering=False, debug=False,
                   num_devices=N_CORES)

    enhy_d = nc.dram_tensor("enhy_loc", [BL, S, H], f32, kind="ExternalInput").ap()
    ldehy_d = nc.dram_tensor("last_dehy_loc", [BL, H], f32, kind="ExternalInput").ap()
    past_d = nc.dram_tensor("past_loc", [BL, S], f32, kind="ExternalInput").ap()
    wen_d = nc.dram_tensor("w_enT16", [HC, P, H], f16, kind="ExternalInput").ap()
    wde_d = nc.dram_tensor("w_deT16", [HC, P, H], f16, kind="ExternalInput").ap()
    wcv_d = nc.dram_tensor("w_cvT", [P, OC], f32, kind="ExternalInput").ap()
    wout_d = nc.dram_tensor("w_outT16", [P, OC], f16, kind="ExternalInput").ap()
    bvec_d = nc.dram_tensor("bvecT", [P, OC], f32, kind="ExternalInput").ap()
    id32_d = nc.dram_tensor("id32", [P, BL], f32, kind="ExternalInput").ap()
    id16_d = nc.dram_tensor("id16", [P, BL], f16, kind="ExternalInput").ap()

    hattn_d = nc.dram_tensor("h_attn_loc", [BL, H], f32, kind="ExternalOutput").ap()
    attn_d = nc.dram_tensor("attn_loc", [BL, S], f32, kind="ExternalOutput").ap()

    with tile.TileContext(nc) as tc:
        with (
            tc.tile_pool(name="consts", bufs=1) as consts,
            tc.tile_pool(name="e16s", bufs=1) as e16s,
            tc.tile_pool(name="e32p", bufs=2) as e32p,
            tc.tile_pool(name="eTp", bufs=3) as eTp,
            tc.tile_pool(name="pbp", bufs=2) as pbp,
            tc.tile_pool(name="wtp", bufs=10) as wtp,
            tc.tile_pool(name="rows", bufs=2) as rows,
            tc.tile_pool(name="ps_agg", bufs=3, space="PSUM") as ps_agg,
            tc.tile_pool(name="ps_score", bufs=2, space="PSUM") as ps_score,
            tc.tile_pool(name="ps_misc", bufs=2, space="PSUM") as ps_misc,
        ):
            # ---- constants in ----
            wen_sb = consts.tile([P, HC, H], f16)
            nc.sync.dma_start(wen_sb[:], wen_d.rearrange("t p o -> p t o"))
            wde_sb = consts.tile([P, HC, H], f16)
            nc.sync.dma_start(wde_sb[:], wde_d.rearrange("t p o -> p t o"))
            wcv_sb = consts.tile([P, OC], f32)
            nc.sync.dma_start(wcv_sb[:], wcv_d)
            wout_sb = consts.tile([P, OC], f16)
            nc.sync.dma_start(wout_sb[:], wout_d)
            bvec_sb = consts.tile([P, OC], f32)
            nc.sync.dma_start(bvec_sb[:], bvec_d)
            id32_sb = consts.tile([P, BL], f32)
            nc.sync.dma_start(id32_sb[:], id32_d)
            id16_sb = consts.tile([P, BL], f16)
            nc.sync.dma_start(id16_sb[:], id16_d)

            # ---- last_dehy^T (pad to 128 partitions, transpose via identity mm) ----
            ldpad = consts.tile([P, H], f32)
            nc.vector.memset(ldpad[:], 0.0)
            nc.sync.dma_start(ldpad[0:BL, :], ldehy_d)
            ldT = consts.tile([P, HC, BL], f16)  # [h_in, h_out, b]
            for t in range(HC):
                ps = ps_misc.tile([P, BL], f32, tag="m")
                nc.tensor.matmul(ps[:], ldpad[:, t * P:(t + 1) * P], id32_sb[:],
                                 start=True, stop=True)
                nc.vector.tensor_copy(ldT[:, t, :], ps[:])

            # ---- de_proj = last_dehy @ W_de.T  -> [BL, H] ----
            depad = consts.tile([P, H], f32)
            nc.vector.memset(depad[:], 0.0)
            for oh in range(NH):
                psd = ps_misc.tile([BL, 512], f32, tag="m")
                for t in range(HC):
                    nc.tensor.matmul(psd[:], ldT[:, t, :],
                                     wde_sb[:, t, oh * 512:(oh + 1) * 512],
                                     start=(t == 0), stop=(t == HC - 1))
                nc.vector.tensor_copy(depad[0:BL, oh * 512:(oh + 1) * 512], psd[:])

            # ---- dvecT[o_in, o_out, b] = de_proj^T + (b_en+b_de+b_cv) ----
            dvecT = consts.tile([P, OC, BL], f32)
            for t in range(OC):
                ps = ps_misc.tile([P, BL], f32, tag="m")
                nc.tensor.matmul(ps[:], depad[:, t * P:(t + 1) * P], id32_sb[:],
                                 start=True, stop=True)
                nc.vector.tensor_scalar_add(dvecT[:, t, :], ps[:],
                                            bvec_sb[:, t:t + 1])

            # ---- per-batch main pipeline (software-pipelined: prep runs
            #      two batches ahead so engine FIFOs never invert) ----
            e16_tiles = []
            eT_tiles = {}
            pb_tiles = {}
            scores_sb = consts.tile([BL, S], f32)

            def prep(b):
                e16 = e16s.tile([P, SC, H], f16, tag=f"e16_{b}")
                e16_tiles.append(e16)
                e32 = e32p.tile([P, SC, H], f32)
                nc.sync.dma_start(
                    e32[:], enhy_d[b].rearrange("(so p) h -> p so h", p=P))
                eT = eTp.tile([P, HC, S], f16)  # [h_in, h_out, s]
                eT_tiles[b] = eT
                for so in range(SC):
                    if so % 2 == 0:
                        nc.vector.tensor_copy(e16[:, so, :], e32[:, so, :])
                    else:
                        nc.scalar.copy(e16[:, so, :], e32[:, so, :])
                    nc.sync.dma_start_transpose(
                        eT[:, :, so * P:(so + 1) * P], e16[:, so, :])
                pb = pbp.tile([P, S], f32)
                pb_tiles[b] = pb
                nc.sync.dma_start(pb[:], past_d[b].partition_broadcast(P))

            def compute(b):
                eT, pb = eT_tiles[b], pb_tiles[b]
                wts = []
                for oc in range(OC):
                    ps = ps_agg.tile([P, S], f32)
                    for hc in range(HC):
                        nc.tensor.matmul(ps[:], wen_sb[:, hc, oc * P:(oc + 1) * P],
                                         eT[:, hc, :],
                                         start=(hc == 0), stop=(hc == HC - 1))
                    # agg += past[b,s] * W_cv[o]
                    nc.vector.scalar_tensor_tensor(
                        out=ps[:], in0=pb[:], scalar=wcv_sb[:, oc:oc + 1],
                        in1=ps[:], op0=Alu.mult, op1=Alu.add)
                    # wt = tanh(agg + de_proj[b,o] + bias[o])
                    wt = wtp.tile([P, S], f16)
                    wts.append(wt)
                    nc.scalar.activation(out=wt[:], in_=ps[:], func=Act.Tanh,
                                         bias=dvecT[:, oc, b:b + 1], scale=1.0)
                # score MMs after all main MMs: they never block the PE stream
                score_ps = ps_score.tile([1, S], f32)
                for oc in range(OC):
                    nc.tensor.matmul(score_ps[:], wout_sb[:, oc:oc + 1],
                                     wts[oc][:],
                                     start=(oc == 0), stop=(oc == OC - 1),
                                     skip_group_check=True)
                srow = rows.tile([1, S], f32, tag="srow")
                nc.vector.tensor_copy(srow[:], score_ps[:])
                nc.sync.dma_start(scores_sb[b:b + 1, :], srow[:])

            prep(0)
            prep(1)
            for b in range(BL):
                compute(b)
                if b + 2 < BL:
                    prep(b + 2)

            # ---- softmax over s for all batches at once ([BL, S]) ----
            mx = consts.tile([BL, 1], f32)
            nc.vector.tensor_reduce(mx[:], scores_sb[:], axis=mybir.AxisListType.X,
                                    op=Alu.max, negate=True)
            ex = consts.tile([BL, S], f32)
            sm = consts.tile([BL, 1], f32)
            nc.scalar.activation(out=ex[:], in_=scores_sb[:], func=Act.Exp,
                                 bias=mx[:], scale=1.0, accum_out=sm[:])
            rec = consts.tile([BL, 1], f32)
            nc.vector.reciprocal(rec[:], sm[:])
            attn32 = consts.tile([BL, S], f32)
            nc.vector.tensor_scalar_mul(attn32[:], ex[:], rec[:])
            nc.sync.dma_start(attn_d[:], attn32[:])

            # ---- attn^T (fp16) for the final contraction ----
            atpad = consts.tile([P, S], f16)
            nc.vector.memset(atpad[:], 0.0)
            nc.vector.tensor_copy(atpad[0:BL, :], attn32[:])
            attnT = consts.tile([P, SC, BL], f16)  # [s_in, s_out, b]
            for sc in range(SC):
                ps = ps_misc.tile([P, BL], f32, tag="m")
                nc.tensor.matmul(ps[:], atpad[:, sc * P:(sc + 1) * P], id16_sb[:],
                                 start=True, stop=True)
                nc.vector.tensor_copy(attnT[:, sc, :], ps[:])

            # ---- h_attn[b,h] = sum_s attn[b,s] * enhy[b,s,h] ----
            for b in range(BL):
                for hh in range(NH):
                    hp = ps_misc.tile([1, 512], f32, tag="m")
                    for sc in range(SC):
                        nc.tensor.matmul(hp[:], attnT[:, sc, b:b + 1],
                                         e16_tiles[b][:, sc, hh * 512:(hh + 1) * 512],
                                         start=(sc == 0), stop=(sc == SC - 1),
                                         skip_group_check=True)
                    hrow = rows.tile([1, 512], f32, tag="hrow")
                    nc.vector.tensor_copy(hrow[:], hp[:])
                    nc.sync.dma_start(hattn_d[b:b + 1, hh * 512:(hh + 1) * 512],
                                      hrow[:])

    nc.compile()
    return nc


def _get_program():
    global _PROGRAM
    if _PROGRAM is None:
        _PROGRAM = _build_program()
    return _PROGRAM


def _host_prep(W_en, b_en, W_de, b_de, W_cv, b_cv, W_out):
    """One-time weight layout transforms (transpose/cast/reshape)."""
    w_enT16 = np.ascontiguousarray(
        W_en.T.astype(np.float16).reshape(HC, P, H))
    w_deT16 = np.ascontiguousarray(
        W_de.T.astype(np.float16).reshape(HC, P, H))
    w_cvT = np.ascontiguousarray(W_cv[:, 0].reshape(OC, P).T.astype(np.float32))
    w_outT16 = np.ascontiguousarray(W_out[0].reshape(OC, P).T.astype(np.float16))
    bvecT = np.ascontiguousarray(
        (b_en + b_de + b_cv).reshape(OC, P).T.astype(np.float32))
    id32 = np.eye(P, BL, dtype=np.float32)
    id16 = np.eye(P, BL, dtype=np.float16)
    return dict(w_enT16=w_enT16, w_deT16=w_deT16, w_cvT=w_cvT,
                w_outT16=w_outT16, bvecT=bvecT, id32=id32, id16=id16)


def make_in_maps(inputs):
    enhy = np.ascontiguousarray(np.asarray(inputs["enhy"], dtype=np.float32))
    last_dehy = np.ascontiguousarray(np.asarray(inputs["last_dehy"], np.float32))
    past_attn = np.ascontiguousarray(np.asarray(inputs["past_attn"], np.float32))
    w = _host_prep(*(np.asarray(inputs[k], np.float32) for k in
                     ("W_en", "b_en", "W_de", "b_de", "W_cv", "b_cv", "W_out")))
    in_maps = []
    for c in range(N_CORES):
        sl = slice(c * BL, (c + 1) * BL)
        in_maps.append({
            "enhy_loc": enhy[sl],
            "last_dehy_loc": last_dehy[sl],
            "past_loc": past_attn[sl],
            **w,
        })
    return in_maps


def kernel(last_dehy, enhy, past_attn, hidden_attn,
           W_en, b_en, W_de, b_de, W_cv, b_cv, W_out):
    from concourse.bass_utils import run_bass_kernel_spmd

    nc = _get_program()
    in_maps = make_in_maps(dict(
        last_dehy=last_dehy, enhy=enhy, past_attn=past_attn,
        W_en=W_en, b_en=b_en, W_de=W_de, b_de=b_de,
        W_cv=W_cv, b_cv=b_cv, W_out=W_out))
    res = run_bass_kernel_spmd(nc, in_maps, core_ids=list(range(N_CORES)))
    h_attn = np.concatenate([res.results[c]["h_attn_loc"] for c in range(N_CORES)])
    attn = np.concatenate([res.results[c]["attn_loc"] for c in range(N_CORES)])
    hidden = np.array(np.asarray(hidden_attn, np.float32))
    return (h_attn, attn, hidden)


# revision 13
# speedup vs baseline: 1.3337x; 1.0121x over previous
"""Bahdanau attention kernel for Trainium2, data-parallel over batch on 8 cores.

Shapes (hardcoded): B=64, S=512, H=1024, AH=512. Each core handles 8 batches.

Math per batch b:
  en_proj[s,o] = sum_h enhy[b,s,h] * W_en[o,h]            (big matmul, fp16)
  agg[s,o]     = en_proj + de_proj[b,o] + b_* + past[b,s]*W_cv[o]
  score[s]     = sum_o tanh(agg[s,o]) * W_out[o]
  attn         = softmax_s(score);  h_attn[h] = sum_s attn[s]*enhy[b,s,h]

Device layout: agg is computed transposed ([o on partitions, s free]) so the
W_out contraction is a matmul; de_proj+bias rides in as the tanh bias
([P,1] per (o-chunk, b)); the rank-1 past*W_cv term is one fused DVE op.
"""

import numpy as np

B, S, H = 64, 512, 1024
N_CORES = 8
BL = B // N_CORES  # batches per core = 8
P = 128
HC = H // P        # h chunks = 8
OC = H // P        # o chunks = 8
SC = S // P        # s chunks = 4
NH = H // 512      # 512-wide halves of H = 2

_PROGRAM = None


def _build_program():
    import concourse.bass as bass
    import concourse.mybir as mybir
    import concourse.tile as tile
    from concourse import bacc

    f32 = mybir.dt.float32
    f16 = mybir.dt.float16
    Alu = mybir.AluOpType
    Act = mybir.ActivationFunctionType

    nc = bacc.Bacc("TRN2", target_bir_lowering=False, debug=False,
                   num_devices=N_CORES)

    enhy_d = nc.dram_tensor("enhy_loc", [BL, S, H], f32, kind="ExternalInput").ap()
    ldehy_d = nc.dram_tensor("last_dehy_loc", [BL, H], f32, kind="ExternalInput").ap()
    past_d = nc.dram_tensor("past_loc", [BL, S], f32, kind="ExternalInput").ap()
    wen_d = nc.dram_tensor("w_enT16", [HC, P, H], f16, kind="ExternalInput").ap()
    wde_d = nc.dram_tensor("w_deT16", [HC, P, H], f16, kind="ExternalInput").ap()
    wcv_d = nc.dram_tensor("w_cvT", [P, OC], f32, kind="ExternalInput").ap()
    wout_d = nc.dram_tensor("w_outT16", [P, OC], f16, kind="ExternalInput").ap()
    bvec_d = nc.dram_tensor("bvecT", [P, OC], f32, kind="ExternalInput").ap()
    id32_d = nc.dram_tensor("id32", [P, BL], f32, kind="ExternalInput").ap()
    id16_d = nc.dram_tensor("id16", [P, BL], f16, kind="ExternalInput").ap()

    hattn_d = nc.dram_tensor("h_attn_loc", [BL, H], f32, kind="ExternalOutput").ap()
    attn_d = nc.dram_tensor("attn_loc", [BL, S], f32, kind="ExternalOutput").ap()

    with tile.TileContext(nc) as tc:
        with (
            tc.tile_pool(name="consts", bufs=1) as consts,
            tc.tile_pool(name="e16s", bufs=1) as e16s,
            tc.tile_pool(name="e32p", bufs=2) as e32p,
            tc.tile_pool(name="eTp", bufs=3) as eTp,
            tc.tile_pool(name="pbp", bufs=3) as pbp,
            tc.tile_pool(name="wtp", bufs=10) as wtp,
            tc.tile_pool(name="rows", bufs=2) as rows,
            tc.tile_pool(name="ps_agg", bufs=3, space="PSUM") as ps_agg,
            tc.tile_pool(name="ps_score", bufs=2, space="PSUM") as ps_score,
            tc.tile_pool(name="ps_misc", bufs=2, space="PSUM") as ps_misc,
        ):
            # ---- DMA order matters: wde first (feeds dvecT, needed by the
            # first tanh), then batch-0 prep, then wen in per-hc chunks so the
            # first main matmuls can start before the full weight lands.
            wde_sb = consts.tile([P, HC, H], f16)
            nc.sync.dma_start(wde_sb[:], wde_d.rearrange("t p o -> p t o"))
            ldpad = consts.tile([P, H], f32)
            nc.vector.memset(ldpad[:], 0.0)
            nc.sync.dma_start(ldpad[0:BL, :], ldehy_d)
            wcv_sb = consts.tile([P, OC], f32)
            nc.sync.dma_start(wcv_sb[:], wcv_d)
            wout_sb = consts.tile([P, OC], f16)
            nc.sync.dma_start(wout_sb[:], wout_d)
            bvec_sb = consts.tile([P, OC], f32)
            nc.sync.dma_start(bvec_sb[:], bvec_d)
            id32_sb = consts.tile([P, BL], f32)
            nc.sync.dma_start(id32_sb[:], id32_d)
            id16_sb = consts.tile([P, BL], f16)
            nc.sync.dma_start(id16_sb[:], id16_d)

            e16_tiles = []
            eT_tiles = {}
            pb_tiles = {}
            # separate per-half score tiles (compute engines need partition
            # slices based at 0, so batches 4..7 live at partitions 0..3)
            scores_half = [consts.tile([4, S], f32, tag="scA", name="scA"),
                           consts.tile([4, S], f32, tag="scB", name="scB")]

            def prep(b):
                e16 = e16s.tile([P, SC, H], f16, tag=f"e16_{b}")
                e16_tiles.append(e16)
                e32 = e32p.tile([P, SC, H], f32)
                nc.sync.dma_start(
                    e32[:], enhy_d[b].rearrange("(so p) h -> p so h", p=P))
                eT = eTp.tile([P, HC, S], f16)  # [h_in, h_out, s]
                eT_tiles[b] = eT
                for so in range(SC):
                    if so % 2 == 0:
                        nc.vector.tensor_copy(e16[:, so, :], e32[:, so, :])
                    else:
                        nc.scalar.copy(e16[:, so, :], e32[:, so, :])
                    nc.sync.dma_start_transpose(
                        eT[:, :, so * P:(so + 1) * P], e16[:, so, :])
                pb = pbp.tile([P, S], f32)
                pb_tiles[b] = pb
                nc.sync.dma_start(pb[:], past_d[b].partition_broadcast(P))

            prep(0)

            wen_sb = consts.tile([P, HC, H], f16)
            for hc in range(HC):
                nc.sync.dma_start(wen_sb[:, hc, :], wen_d[hc])

            prep(1)

            # ---- last_dehy^T via identity matmuls, then de_proj, then dvecT
            ldT = consts.tile([P, HC, BL], f16)  # [h_in, h_out, b]
            for t in range(HC):
                ps = ps_misc.tile([P, BL], f32, tag="m")
                nc.tensor.matmul(ps[:], ldpad[:, t * P:(t + 1) * P], id32_sb[:],
                                 start=True, stop=True)
                nc.vector.tensor_copy(ldT[:, t, :], ps[:])

            depad = consts.tile([P, H], f32)
            nc.vector.memset(depad[:], 0.0)
            for oh in range(NH):
                psd = ps_misc.tile([BL, 512], f32, tag="m")
                for t in range(HC):
                    nc.tensor.matmul(psd[:], ldT[:, t, :],
                                     wde_sb[:, t, oh * 512:(oh + 1) * 512],
                                     start=(t == 0), stop=(t == HC - 1))
                nc.vector.tensor_copy(depad[0:BL, oh * 512:(oh + 1) * 512], psd[:])

            dvecT = consts.tile([P, OC, BL], f32)
            for t in range(OC):
                ps = ps_misc.tile([P, BL], f32, tag="m")
                nc.tensor.matmul(ps[:], depad[:, t * P:(t + 1) * P], id32_sb[:],
                                 start=True, stop=True)
                nc.vector.tensor_scalar_add(dvecT[:, t, :], ps[:],
                                            bvec_sb[:, t:t + 1])

            prep(2)

            def compute(b):
                eT, pb = eT_tiles[b], pb_tiles[b]
                wts = []
                for oc in range(OC):
                    ps = ps_agg.tile([P, S], f32)
                    for hc in range(HC):
                        nc.tensor.matmul(ps[:], wen_sb[:, hc, oc * P:(oc + 1) * P],
                                         eT[:, hc, :],
                                         start=(hc == 0), stop=(hc == HC - 1))
                    # agg += past[b,s] * W_cv[o]
                    nc.vector.scalar_tensor_tensor(
                        out=ps[:], in0=pb[:], scalar=wcv_sb[:, oc:oc + 1],
                        in1=ps[:], op0=Alu.mult, op1=Alu.add)
                    # wt = tanh(agg + de_proj[b,o] + bias[o])
                    wt = wtp.tile([P, S], f16)
                    wts.append(wt)
                    nc.scalar.activation(out=wt[:], in_=ps[:], func=Act.Tanh,
                                         bias=dvecT[:, oc, b:b + 1], scale=1.0)
                # score MMs after all main MMs: they never block the PE stream
                score_ps = ps_score.tile([1, S], f32)
                for oc in range(OC):
                    nc.tensor.matmul(score_ps[:], wout_sb[:, oc:oc + 1],
                                     wts[oc][:],
                                     start=(oc == 0), stop=(oc == OC - 1),
                                     skip_group_check=True)
                srow = rows.tile([1, S], f32, tag="srow")
                nc.vector.tensor_copy(srow[:], score_ps[:])
                half, r = divmod(b, 4)
                nc.sync.dma_start(scores_half[half][r:r + 1, :], srow[:])

            attnT = consts.tile([P, SC, BL], f16)  # [s_in, s_out, b]

            def softmax_half(h):  # h in (0, 1): batches 4h..4h+3
                lo, hi = 4 * h, 4 * h + 4
                sc_sb = scores_half[h]
                mx = consts.tile([4, 1], f32, tag=f"mx{h}")
                nc.vector.tensor_reduce(mx[:], sc_sb[:],
                                        axis=mybir.AxisListType.X,
                                        op=Alu.max, negate=True)
                ex = consts.tile([4, S], f32, tag=f"ex{h}")
                sm = consts.tile([4, 1], f32, tag=f"sm{h}")
                nc.scalar.activation(out=ex[:], in_=sc_sb[:],
                                     func=Act.Exp, bias=mx[:], scale=1.0,
                                     accum_out=sm[:])
                rec = consts.tile([4, 1], f32, tag=f"rec{h}")
                nc.vector.reciprocal(rec[:], sm[:])
                attn32 = consts.tile([4, S], f32, tag=f"attn32{h}")
                nc.vector.tensor_scalar_mul(attn32[:], ex[:], rec[:])
                nc.sync.dma_start(attn_d[lo:hi, :], attn32[:])
                # attn^T fp16: rows 0..3 of a zeroed pad tile hold this
                # half's batches; identity columns 0..3 select them and the
                # result lands in this half's slots of attnT
                atpad = consts.tile([P, S], f16, tag=f"atpad{h}")
                nc.vector.memset(atpad[:], 0.0)
                nc.vector.tensor_copy(atpad[0:4, :], attn32[:])
                for sc in range(SC):
                    ps = ps_misc.tile([P, BL], f32, tag="m")
                    nc.tensor.matmul(ps[:], atpad[:, sc * P:(sc + 1) * P],
                                     id16_sb[:], start=True, stop=True)
                    nc.vector.tensor_copy(attnT[:, sc, lo:hi], ps[:, 0:4])

            def final(b):
                for hh in range(NH):
                    hp = ps_misc.tile([1, 512], f32, tag="m")
                    for sc in range(SC):
                        nc.tensor.matmul(hp[:], attnT[:, sc, b:b + 1],
                                         e16_tiles[b][:, sc, hh * 512:(hh + 1) * 512],
                                         start=(sc == 0), stop=(sc == SC - 1),
                                         skip_group_check=True)
                    hrow = rows.tile([1, 512], f32, tag="hrow")
                    nc.vector.tensor_copy(hrow[:], hp[:])
                    nc.sync.dma_start(hattn_d[b:b + 1, hh * 512:(hh + 1) * 512],
                                      hrow[:])

            for b in range(BL):
                compute(b)
                if b >= 1 and b + 2 < BL:
                    prep(b + 2)
                if b == 4:
                    softmax_half(0)
                if b == 5:
                    for bb in range(4):
                        final(bb)
            softmax_half(1)
            for bb in range(4, BL):
                final(bb)

    nc.compile()
    return nc


def _get_program():
    global _PROGRAM
    if _PROGRAM is None:
        _PROGRAM = _build_program()
    return _PROGRAM


def _host_prep(W_en, b_en, W_de, b_de, W_cv, b_cv, W_out):
    """One-time weight layout transforms (transpose/cast/reshape)."""
    w_enT16 = np.ascontiguousarray(
        W_en.T.astype(np.float16).reshape(HC, P, H))
    w_deT16 = np.ascontiguousarray(
        W_de.T.astype(np.float16).reshape(HC, P, H))
    w_cvT = np.ascontiguousarray(W_cv[:, 0].reshape(OC, P).T.astype(np.float32))
    w_outT16 = np.ascontiguousarray(W_out[0].reshape(OC, P).T.astype(np.float16))
    bvecT = np.ascontiguousarray(
        (b_en + b_de + b_cv).reshape(OC, P).T.astype(np.float32))
    id32 = np.eye(P, BL, dtype=np.float32)
    id16 = np.eye(P, BL, dtype=np.float16)
    return dict(w_enT16=w_enT16, w_deT16=w_deT16, w_cvT=w_cvT,
                w_outT16=w_outT16, bvecT=bvecT, id32=id32, id16=id16)


def make_in_maps(inputs):
    enhy = np.ascontiguousarray(np.asarray(inputs["enhy"], dtype=np.float32))
    last_dehy = np.ascontiguousarray(np.asarray(inputs["last_dehy"], np.float32))
    past_attn = np.ascontiguousarray(np.asarray(inputs["past_attn"], np.float32))
    w = _host_prep(*(np.asarray(inputs[k], np.float32) for k in
                     ("W_en", "b_en", "W_de", "b_de", "W_cv", "b_cv", "W_out")))
    in_maps = []
    for c in range(N_CORES):
        sl = slice(c * BL, (c + 1) * BL)
        in_maps.append({
            "enhy_loc": enhy[sl],
            "last_dehy_loc": last_dehy[sl],
            "past_loc": past_attn[sl],
            **w,
        })
    return in_maps


def kernel(last_dehy, enhy, past_attn, hidden_attn,
           W_en, b_en, W_de, b_de, W_cv, b_cv, W_out):
    from concourse.bass_utils import run_bass_kernel_spmd

    nc = _get_program()
    in_maps = make_in_maps(dict(
        last_dehy=last_dehy, enhy=enhy, past_attn=past_attn,
        W_en=W_en, b_en=b_en, W_de=W_de, b_de=b_de,
        W_cv=W_cv, b_cv=b_cv, W_out=W_out))
    res = run_bass_kernel_spmd(nc, in_maps, core_ids=list(range(N_CORES)))
    h_attn = np.concatenate([res.results[c]["h_attn_loc"] for c in range(N_CORES)])
    attn = np.concatenate([res.results[c]["attn_loc"] for c in range(N_CORES)])
    hidden = np.array(np.asarray(hidden_attn, np.float32))
    return (h_attn, attn, hidden)
